# revision 4
# baseline (speedup 1.0000x reference)
"""BasesDecomposition GNN message passing on 8 Trainium2 NeuronCores.

Math (reference):
    seg  = edge_type * N + target
    h    = segment_sum(x[source] * ew, seg)        # (R, N, D)
    out  = einsum('rb,bio,rni->no', bw, bases, h)  # (N, D)

Key algebraic restructuring: fold the relation->basis projection into a
per-edge coefficient vector  c_e[b] = bw[edge_type_e, b] * ew_e  so the
accumulator shrinks from (R,N,D) to (B,N,D):
    g[b, n, i] = sum_{e: tgt_e = n} c_e[b] * x[src_e, i]
    out[n, o]  = sum_b sum_i g[b, n, i] * bases[b, i, o]

Sharding: nodes by target-id range across the 8 cores (no all-reduce);
each core consumes only the edges targeting its node range. Edges are
sorted by target on the host and packed into 128-edge tiles per 128-node
"node tile", padded with null edges (ew=0 -> c=0).

Device kernel per (node-tile, edge-tile):
    - indirect-DMA gather of x rows by source index        (128 x 128 f32)
    - indirect-DMA gather of bw rows by edge type          (128 x 4)
    - c = bw_rows * ew                                     (DVE)
    - selector S[e, b*M+m] = (iota[m]==tgt_rel_e) * c_e[b] (DVE fused ts)
    - PSUM accumulate: g_T[i, b*M+m] += xg^T @ S           (PE matmul)
then per node-tile: out[m, o] = sum_b g_T[:, bM..]^T @ bases[b] (PE), DMA out.
"""

import numpy as np

import concourse.bass as bass
import concourse.mybir as mybir
import concourse.tile as tile
from concourse import bacc
from concourse.bass_utils import run_bass_kernel_spmd

NCORES = 8
P = 128          # edges per tile (matmul contraction dim)
M = 128          # nodes per node-tile (selector block width)

TRACE = False
LAST_PROFILE = None

_PROG_CACHE = {}


def _build_program(N, D, R, B, NPC, NT, T):
    fp = mybir.dt.float32
    i32 = mybir.dt.int32

    nc = bacc.Bacc("TRN2", target_bir_lowering=False, debug=False, num_devices=NCORES)
    x_d = nc.dram_tensor("x", [N, D], fp, kind="ExternalInput").ap()
    bases_d = nc.dram_tensor("bases", [B, D, D], fp, kind="ExternalInput").ap()
    bw_d = nc.dram_tensor("bw", [R, B], fp, kind="ExternalInput").ap()
    iota_d = nc.dram_tensor("iota", [P, M], fp, kind="ExternalInput").ap()
    mi_d = nc.dram_tensor("meta_i", [NT, P, 2 * T], i32, kind="ExternalInput").ap()
    mf_d = nc.dram_tensor("meta_f", [NT, P, 2 * T], fp, kind="ExternalInput").ap()
    out_d = nc.dram_tensor("out", [NPC, D], fp, kind="ExternalOutput").ap()

    with tile.TileContext(nc) as tc:
        with (
            tc.tile_pool(name="const", bufs=1) as constp,
            tc.tile_pool(name="meta", bufs=3) as metap,
            tc.tile_pool(name="xg", bufs=8) as xgp,
            tc.tile_pool(name="sel", bufs=4) as selp,
            tc.tile_pool(name="small", bufs=8) as smallp,
            tc.tile_pool(name="gsb", bufs=2) as gsbp,
            tc.tile_pool(name="osb", bufs=3) as osbp,
            tc.tile_pool(name="psg", bufs=2, space="PSUM") as psgp,
            tc.tile_pool(name="pso", bufs=2, space="PSUM") as psop,
        ):
            iota_sb = constp.tile([P, M], fp)
            nc.sync.dma_start(out=iota_sb[:], in_=iota_d[:])
            bases_sb = constp.tile([P, B * D], fp)
            for b in range(B):
                nc.sync.dma_start(out=bases_sb[:, b * D:(b + 1) * D], in_=bases_d[b])

            for nt in range(NT):
                m_lo = nt * M
                m_sz = min(M, NPC - m_lo)

                mi = metap.tile([P, 2 * T], i32)
                mf = metap.tile([P, 2 * T], fp)
                nc.sync.dma_start(out=mi[:], in_=mi_d[nt])
                nc.sync.dma_start(out=mf[:], in_=mf_d[nt])

                pg = psgp.tile([P, B * M], fp)
                for t in range(T):
                    xg = xgp.tile([P, D], fp)
                    nc.gpsimd.indirect_dma_start(
                        out=xg[:],
                        out_offset=None,
                        in_=x_d[:],
                        in_offset=bass.IndirectOffsetOnAxis(
                            ap=mi[:, 2 * t:2 * t + 1], axis=0
                        ),
                    )
                    wg = smallp.tile([P, B], fp)
                    nc.gpsimd.indirect_dma_start(
                        out=wg[:],
                        out_offset=None,
                        in_=bw_d[:],
                        in_offset=bass.IndirectOffsetOnAxis(
                            ap=mi[:, 2 * t + 1:2 * t + 2], axis=0
                        ),
                    )
                    c = smallp.tile([P, B], fp)
                    nc.vector.tensor_scalar_mul(c[:], wg[:], mf[:, 2 * t + 1:2 * t + 2])

                    s4 = selp.tile([P, B * M], fp)
                    for b in range(B):
                        nc.vector.tensor_scalar(
                            s4[:, b * M:(b + 1) * M],
                            iota_sb[:],
                            mf[:, 2 * t:2 * t + 1],
                            c[:, b:b + 1],
                            mybir.AluOpType.is_equal,
                            mybir.AluOpType.mult,
                        )
                    nc.tensor.matmul(
                        out=pg[:],
                        lhsT=xg[:],
                        rhs=s4[:],
                        start=(t == 0),
                        stop=(t == T - 1),
                    )

                gsb = gsbp.tile([P, B * M], fp)
                nc.vector.tensor_copy(out=gsb[:], in_=pg[:])

                po = psop.tile([P, D], fp)
                for b in range(B):
                    nc.tensor.matmul(
                        out=po[:m_sz, :],
                        lhsT=gsb[:, b * M:b * M + m_sz],
                        rhs=bases_sb[:, b * D:(b + 1) * D],
                        start=(b == 0),
                        stop=(b == B - 1),
                    )
                osb = osbp.tile([P, D], fp)
                nc.vector.tensor_copy(out=osb[:m_sz, :], in_=po[:m_sz, :])
                nc.sync.dma_start(out=out_d[m_lo:m_lo + m_sz, :], in_=osb[:m_sz, :])
    nc.compile()
    return nc


def kernel(x, source, target, edge_type, edge_weights, base_weights, bases):
    global LAST_PROFILE
    x = np.ascontiguousarray(np.asarray(x), dtype=np.float32)
    src = np.asarray(source).astype(np.int32)
    tgt = np.asarray(target).astype(np.int32)
    et = np.asarray(edge_type).astype(np.int32)
    ew = np.ascontiguousarray(np.asarray(edge_weights), dtype=np.float32)
    bw = np.ascontiguousarray(np.asarray(base_weights), dtype=np.float32)
    bs = np.ascontiguousarray(np.asarray(bases), dtype=np.float32)

    N, D = x.shape
    R, B = bw.shape
    E = src.shape[0]
    NPC = N // NCORES
    NT = (NPC + M - 1) // M

    # ---- host-side sharding: sort edges by target, pack per node-tile ----
    order = np.argsort(tgt, kind="stable")
    src_s = src[order]
    tgt_s = tgt[order]
    et_s = et[order]
    ew_s = ew[order]

    core = tgt_s // NPC
    local = tgt_s - core * NPC
    ntile = local // M
    tgtf = (local - ntile * M).astype(np.float32)

    gid = core * NT + ntile
    counts = np.bincount(gid, minlength=NCORES * NT)
    T = int(np.ceil(counts.max() / P))
    cap = T * P

    starts = np.zeros(NCORES * NT + 1, dtype=np.int64)
    np.cumsum(counts, out=starts[1:])
    pos = np.arange(E, dtype=np.int64) - starts[gid]
    slot = gid * cap + pos

    meta_i = np.zeros((NCORES * NT * cap, 2), dtype=np.int32)
    meta_f = np.zeros((NCORES * NT * cap, 2), dtype=np.float32)
    meta_i[slot, 0] = src_s
    meta_i[slot, 1] = et_s
    meta_f[slot, 0] = tgtf
    meta_f[slot, 1] = ew_s
    # (C, NT, T, P, 2) -> (C, NT, P, T*2) so each partition reads 2T contiguous
    meta_i = np.ascontiguousarray(
        meta_i.reshape(NCORES, NT, T, P, 2).transpose(0, 1, 3, 2, 4)
    ).reshape(NCORES, NT, P, 2 * T)
    meta_f = np.ascontiguousarray(
        meta_f.reshape(NCORES, NT, T, P, 2).transpose(0, 1, 3, 2, 4)
    ).reshape(NCORES, NT, P, 2 * T)

    iota_arr = np.ascontiguousarray(
        np.broadcast_to(np.arange(M, dtype=np.float32), (P, M))
    )

    key = (N, D, R, B, NPC, NT, T)
    if key not in _PROG_CACHE:
        _PROG_CACHE[key] = _build_program(*key)
    nc = _PROG_CACHE[key]

    in_maps = [
        dict(
            x=x,
            bases=bs,
            bw=bw,
            iota=iota_arr,
            meta_i=meta_i[c],
            meta_f=meta_f[c],
        )
        for c in range(NCORES)
    ]
    res = run_bass_kernel_spmd(nc, in_maps, list(range(NCORES)), trace=TRACE)
    LAST_PROFILE = res
    out = np.concatenate([res.results[c]["out"] for c in range(NCORES)], axis=0)
    return out
